# revision 17
# baseline (speedup 1.0000x reference)
"""Trainium2 Bass kernel for MHSA with relative-position bias.

Reference computation (per sample, C=256, N=48*48=2304):
  q = Wq x + bq ; k = Wk x + bk ; v = Wv x + bv        (1x1 convs == channel matmuls)
  L = q^T k + pos^T q          with pos = (rel_h + rel_w).reshape(C, N)
  att = softmax(L, axis=-1) ;  out = v @ att^T

Kernel strategy (data-parallel over batch, 2 samples per core on 8 cores):
  - Logits computed TRANSPOSED: L^T[m,n] = sum_c [k;q][c,m]*[q;pos][c,n]
    (stationary = k/q chunks, moving = q/pos). exp(L^T - 120) lands in
    SBUF already in the [m, n] layout the AV matmul needs - no PE
    transposes and no transpose-copy DVE traffic.
  - slice-outer schedule: each n-slice's softmax denominator (ones-vector
    matmul over P^T = cross-partition sum on the PE) completes after 1/5 of
    the logits, so recip/broadcast/normalize (DVE+GPSIMD) hide under the
    next slice's logits.
  - P8 = e4m3(P^T * 128/rowsum) on DVE (the 128 keeps flat-attention rows
    out of fp8-subnormal flush); v^T evacuated as e4m3 hi + residual lo.
  - AV in fp8 DoubleRow pairs: out = sum_j vhi[j]^T P8[j] + vlo[j]^T P8[j]
    with (2j,2j+1) m-tile pairs per instruction - ~2x PE throughput vs fp16
    at fp16-grade v accuracy (end-to-end rel err ~1.4e-2 vs the 2e-2 gate).
  - HW cadence is gated by the matmul->PSUM->evac round-trip (~550ns dead
    time per evac event, measured), so evacuations are PAIRED: two 512-col
    chains land in adjacent PSUM banks and drain with ONE [128,1024]
    activation read spanning both banks.
"""
import numpy as np
from contextlib import ExitStack

import concourse.bass as bass
import concourse.mybir as mybir
import concourse.tile as tile
from concourse import bacc
from concourse.bass import ds, ts
from concourse.bass_utils import run_bass_kernel_spmd

f32 = mybir.dt.float32
fp16 = mybir.dt.float16
bf16 = mybir.dt.bfloat16
fp8 = mybir.dt.float8e4

B, C, H, W = 16, 256, 48, 48
N = H * W                      # 2304
NCORES = 8
SPC = B // NCORES              # samples per core
NT = N // 128                  # 18 m-tiles
M_SLICES = [(0, 512), (512, 512), (1024, 512), (1536, 512), (2048, 256)]
S_PAIRS = [(0, 1), (2, 3), (4,)]   # slice pairs sharing one wide evac
SHIFT = -120.0                 # softmax stabilizer: logits range [-193, 193]
PSCALE = 128.0                 # fp8 headroom scale for normalized P


def build(loop_n: int = 0, phases: str = "full", loop_xout: bool = False, lag: int = 2):
    nc = bacc.Bacc("TRN2", target_bir_lowering=False, debug=False)

    x_d = nc.dram_tensor("x", [SPC, C, N], fp16, kind="ExternalInput")
    wq_d = nc.dram_tensor("wqT", [C, C], fp16, kind="ExternalInput")
    wk_d = nc.dram_tensor("wkT", [C, C], fp16, kind="ExternalInput")
    wv_d = nc.dram_tensor("wvT", [C, C], fp16, kind="ExternalInput")
    pos_d = nc.dram_tensor("pos", [C, N], fp16, kind="ExternalInput")
    bq_d = nc.dram_tensor("bq", [2, 128, 1], f32, kind="ExternalInput")
    bk_d = nc.dram_tensor("bk", [2, 128, 1], f32, kind="ExternalInput")
    bv_d = nc.dram_tensor("bv", [2, 128, 1], f32, kind="ExternalInput")
    out_d = nc.dram_tensor("out", [SPC, C, N], f32, kind="ExternalOutput")

    with tile.TileContext(nc) as tc, ExitStack() as ctx:
        const = ctx.enter_context(tc.tile_pool(name="const", bufs=1))
        sb = ctx.enter_context(tc.tile_pool(name="sb", bufs=2))
        sb1 = ctx.enter_context(tc.tile_pool(name="sb1", bufs=1))
        ps = ctx.enter_context(tc.tile_pool(name="ps", bufs=1, space="PSUM"))

        wq = [const.tile([128, C], fp16, tag=f"wq{cc}", name=f"wq{cc}") for cc in range(2)]
        wk = [const.tile([128, C], fp16, tag=f"wk{cc}", name=f"wk{cc}") for cc in range(2)]
        wv = [const.tile([128, C], fp16, tag=f"wv{cc}", name=f"wv{cc}") for cc in range(2)]
        for cc in range(2):
            nc.gpsimd.dma_start(wq[cc][:], wq_d.ap()[ds(cc * 128, 128)])
            nc.gpsimd.dma_start(wk[cc][:], wk_d.ap()[ds(cc * 128, 128)])
            nc.gpsimd.dma_start(wv[cc][:], wv_d.ap()[ds(cc * 128, 128)])
        pos = [const.tile([128, N], fp16, tag=f"pos{cc}", name=f"pos{cc}") for cc in range(2)]
        for cc in range(2):
            # pos isn't needed until the logits phase; keep it off the queue
            # that feeds x/weights so projections can start sooner.
            nc.scalar.dma_start(pos[cc][:, 0:1152], pos_d.ap()[ds(cc * 128, 128), ds(0, 1152)])
            nc.scalar.dma_start(pos[cc][:, 1152:N], pos_d.ap()[ds(cc * 128, 128), ds(1152, N - 1152)])
        shift_sb = const.tile([128, 1], f32)
        nc.gpsimd.memset(shift_sb[:], SHIFT)
        ones_sb = const.tile([128, 1], bf16)
        nc.gpsimd.memset(ones_sb[:], 1.0)
        bq_sb = const.tile([128, 2], f32)
        bk_sb = const.tile([128, 2], f32)
        bv_sb = const.tile([128, 2], f32)
        for ot in range(2):
            nc.sync.dma_start(bq_sb[:, ds(ot, 1)], bq_d.ap()[ot])
            nc.sync.dma_start(bk_sb[:, ds(ot, 1)], bk_d.ap()[ot])
            nc.sync.dma_start(bv_sb[:, ds(ot, 1)], bv_d.ap()[ot])

        pre_x = None
        if loop_xout:
            pre_x = {}
            for s in range(SPC):
                for cc in range(2):
                    xt = const.tile([128, N], fp16, tag=f"px{s}{cc}", name=f"px{s}{cc}")
                    nc.sync.dma_start(xt[:], x_d.ap()[s, ds(cc * 128, 128)])
                    pre_x[(s, cc)] = xt

        def wide_ps(name):
            return ps.tile([128, 1024], f32, tag="L", bufs=2, name=name)

        def emit_proj(rep, s):
            """Load x; compute q,k [c,n] fp16; v^T as fp8 hi/lo pairs."""
            xc = []
            for cc in range(2):
                if pre_x is not None:
                    xc.append(pre_x[(s, cc)])
                    continue
                xt = sb.tile([128, N], fp16, tag=f"x{cc}", name=f"x{cc}_{rep}_{s}")
                nc.sync.dma_start(xt[:, 0:1152], x_d.ap()[s, ds(cc * 128, 128), ds(0, 1152)])
                nc.gpsimd.dma_start(xt[:, 1152:N], x_d.ap()[s, ds(cc * 128, 128), ds(1152, N - 1152)])
                xc.append(xt)

            qk = {}
            for pname, wt, bias in (("q", wq, bq_sb), ("k", wk, bk_sb)):
                dst = []
                for ot in range(2):
                    t = sb1.tile([128, N], fp16, tag=f"{pname}{ot}",
                                 name=f"{pname}{ot}_{rep}_{s}")
                    dst.append(t)
                for ot in range(2):
                    for pr in S_PAIRS:
                        pj = wide_ps(f"pj_{rep}_{s}_{pname}{ot}_{pr[0]}")
                        off = 0
                        for mi in pr:
                            mo, mw = M_SLICES[mi]
                            for cc in range(2):
                                nc.tensor.matmul(
                                    pj[:, ds(off, mw)],
                                    wt[cc][:, ds(ot * 128, 128)],
                                    xc[cc][:, ds(mo, mw)],
                                    start=(cc == 0), stop=(cc == 1),
                                )
                            off += mw
                        mo0 = M_SLICES[pr[0]][0]
                        nc.scalar.activation(
                            dst[ot][:, ds(mo0, off)], pj[:, :off],
                            mybir.ActivationFunctionType.Identity,
                            bias=bias[:, ds(ot, 1)], scale=1.0,
                        )
                qk[pname] = dst

            vhi = sb.tile([128, NT, C], fp8, tag="vhi", name=f"vhi_{rep}_{s}")
            vlo = sb.tile([128, NT, C], fp8, tag="vlo", name=f"vlo_{rep}_{s}")
            for tp in range(NT // 2):
                pv = wide_ps(f"pv_{rep}_{s}_{tp}")
                for h in range(2):
                    for cc in range(2):
                        nc.tensor.matmul(
                            pv[:, ds(h * C, C)],
                            xc[cc][:, ds((2 * tp + h) * 128, 128)],
                            wv[cc][:],
                            start=(cc == 0), stop=(cc == 1),
                        )
                nc.scalar.copy(vhi[:, ds(2 * tp, 2), :], pv[:, 0:2 * C])
                nc.vector.tensor_tensor(vlo[:, ds(2 * tp, 2), :], pv[:, 0:2 * C],
                                        vhi[:, ds(2 * tp, 2), :],
                                        mybir.AluOpType.subtract)
            return qk["q"], qk["k"], vhi, vlo

        def body(rep):
            for s in range(SPC):
                q, k, vhi, vlo = emit_proj(rep, s)

                A_ch = [q[0], q[1], pos[0], pos[1]]   # moving   [c, n]
                B_ch = [k[0], k[1], q[0], q[1]]       # stationary [c, m]
                # one shared rowsum bank; consecutive slices alternate offsets
                rs_all = ps.tile([64, 512], f32, tag="rs", bufs=1,
                                 name=f"rs_{rep}_{s}")

                def emit_slice(mi, s=s, A_ch=A_ch, B_ch=B_ch, rs_all=rs_all):
                    """Logits+exp+rowsum+normalize for one n-slice; returns p8."""
                    mo, mw = M_SLICES[mi]
                    Pt = sb1.tile([128, NT, 512], bf16, tag="P", bufs=2,
                                  name=f"P{mi}_{rep}_{s}")
                    rs_ps = rs_all[ds(32 * (mi % 2), 1), :mw]
                    for mp in range(NT // 2):
                        lp = wide_ps(f"lp_{rep}_{s}_{mp}_{mi}")
                        for h in range(2):
                            mt = 2 * mp + h
                            for ci in range(4):
                                nc.tensor.matmul(
                                    lp[:, ds(h * mw, mw)],
                                    B_ch[ci][:, ds(mt * 128, 128)],
                                    A_ch[ci][:, ds(mo, mw)],
                                    start=(ci == 0), stop=(ci == 3),
                                )
                        nc.scalar.activation(
                            Pt[:, ds(2 * mp, 2), :mw], lp[:, 0:2 * mw],
                            mybir.ActivationFunctionType.Exp,
                            bias=shift_sb[:], scale=1.0,
                        )
                        if phases != "logits":
                            for h in range(2):
                                mt = 2 * mp + h
                                nc.tensor.matmul(
                                    rs_ps, ones_sb[:], Pt[:, mt, :mw],
                                    start=(mt == 0), stop=(mt == NT - 1),
                                )
                    if phases == "logits":
                        return None

                    rr = sb1.tile([1, 512], f32, tag=f"rr{mi}", name=f"rr{mi}_{rep}_{s}")
                    nc.vector.reciprocal(rr[:, :mw], rs_ps)
                    nc.vector.tensor_scalar_mul(rr[:, :mw], rr[:, :mw], PSCALE)
                    rb = sb1.tile([128, mw], f32, tag=f"rb{mi}", name=f"rb{mi}_{rep}_{s}")
                    nc.gpsimd.partition_broadcast(rb[:], rr[:, :mw])
                    p8 = sb1.tile([128, NT, 512], fp8, tag="p8", bufs=2,
                                  name=f"p8{mi}_{rep}_{s}")
                    for mt in range(NT):
                        nc.vector.tensor_tensor(
                            p8[:, mt, :mw], Pt[:, mt, :mw], rb[:],
                            mybir.AluOpType.mult,
                        )
                    return p8

                def emit_av_pair(pr, p8s, s=s, vhi=vhi, vlo=vlo):
                    pw = sum(M_SLICES[mi][1] for mi in pr)
                    mo0 = M_SLICES[pr[0]][0]
                    for ct in range(2):
                        po = ps.tile([128, 1024], f32, tag="po", bufs=1,
                                     name=f"po_{rep}_{s}_{pr[0]}_{ct}")
                        off = 0
                        for mi, p8 in zip(pr, p8s):
                            mw = M_SLICES[mi][1]
                            for half, vv in ((0, vhi), (1, vlo)):
                                for j in range(NT // 2):
                                    nc.tensor.matmul(
                                        po[:, ds(off, mw)],
                                        vv[:, ds(2 * j, 2), ds(ct * 128, 128)],
                                        p8[:, ds(2 * j, 2), :mw],
                                        start=(half == 0 and j == 0),
                                        stop=(half == 1 and j == NT // 2 - 1),
                                        perf_mode=mybir.MatmulPerfMode.DoubleRow,
                                    )
                            off += mw
                        oe = sb.tile([128, 1024], f32, tag="oe", bufs=2,
                                     name=f"oe_{rep}_{s}_{pr[0]}_{ct}")
                        nc.scalar.activation(
                            oe[:, :pw], po[:, :pw],
                            mybir.ActivationFunctionType.Identity,
                            bias=bv_sb[:, ds(ct, 1)], scale=1.0 / PSCALE,
                        )
                        dma_eng = nc.sync if ct == 0 else nc.gpsimd
                        dma_eng.dma_start(
                            out_d.ap()[s, ds(ct * 128, 128), ds(mo0, pw)],
                            oe[:, :pw],
                        )

                if phases in ("logits", "noav"):
                    for mi in range(len(M_SLICES)):
                        emit_slice(mi)
                    continue
                # AV for a slice pair is emitted after the following slice's
                # logits so the PE has work while DVE normalizes the pair.
                p8s = {}
                for mi in range(len(M_SLICES)):
                    p8s[mi] = emit_slice(mi)
                    if mi == 2:
                        emit_av_pair(S_PAIRS[0], [p8s[0], p8s[1]])
                    elif mi == 4:
                        emit_av_pair(S_PAIRS[1], [p8s[2], p8s[3]])
                emit_av_pair(S_PAIRS[2], [p8s[4]])

        if loop_n:
            with tc.For_i(0, loop_n, 1):
                body(0)
        else:
            body(0)
    nc.compile()
    return nc


_CACHE = {}


def _get_nc(loop_n: int = 0, phases: str = "full", loop_xout: bool = False, lag: int = 2):
    key = (loop_n, phases, loop_xout, lag)
    if key not in _CACHE:
        _CACHE[key] = build(loop_n, phases, loop_xout, lag)
    return _CACHE[key]


def _make_in_maps(x, Wq, bq, Wk, bk, Wv, bv, rel_h, rel_w):
    f = np.float32
    xr = np.asarray(x, dtype=f).reshape(B, C, N).astype(np.float16)
    pos = (np.asarray(rel_h, dtype=f) + np.asarray(rel_w, dtype=f)).reshape(C, N).astype(np.float16)
    wqT = np.ascontiguousarray(np.asarray(Wq, dtype=f).T).astype(np.float16)
    wkT = np.ascontiguousarray(np.asarray(Wk, dtype=f).T).astype(np.float16)
    wvT = np.ascontiguousarray(np.asarray(Wv, dtype=f).T).astype(np.float16)
    bqr = np.ascontiguousarray(np.asarray(bq, dtype=f).reshape(2, 128, 1))
    bkr = np.ascontiguousarray(np.asarray(bk, dtype=f).reshape(2, 128, 1))
    bvr = np.ascontiguousarray(np.asarray(bv, dtype=f).reshape(2, 128, 1))
    maps = []
    for i in range(NCORES):
        maps.append({
            "x": np.ascontiguousarray(xr[i * SPC:(i + 1) * SPC]),
            "wqT": wqT, "wkT": wkT, "wvT": wvT, "pos": pos,
            "bq": bqr, "bk": bkr, "bv": bvr,
        })
    return maps


def kernel(x, Wq, bq, Wk, bk, Wv, bv, rel_h, rel_w):
    nc = _get_nc()
    in_maps = _make_in_maps(x, Wq, bq, Wk, bk, Wv, bv, rel_h, rel_w)
    res = run_bass_kernel_spmd(nc, in_maps, core_ids=list(range(NCORES)))
    out = np.concatenate([r["out"] for r in res.results], axis=0)
    return np.ascontiguousarray(out.reshape(B, C, H, W).astype(np.float32))


# revision 19
# speedup vs baseline: 1.2546x; 1.2546x over previous
"""Trainium2 Bass kernel for MHSA with relative-position bias.

Reference computation (per sample, C=256, N=48*48=2304):
  q = Wq x + bq ; k = Wk x + bk ; v = Wv x + bv        (1x1 convs == channel matmuls)
  L = q^T k + pos^T q          with pos = (rel_h + rel_w).reshape(C, N)
  att = softmax(L, axis=-1) ;  out = v @ att^T

Kernel strategy (data-parallel over batch, 2 samples per core on 8 cores):
  - Logits computed TRANSPOSED: L^T[m,n] = sum_c [k;q][c,m]*[q;pos][c,n]
    (stationary = k/q chunks, moving = q/pos). exp(L^T - 120) lands in
    SBUF already in the [m, n] layout the AV matmul needs - no PE
    transposes and no transpose-copy DVE traffic.
  - slice-outer schedule: each n-slice's softmax denominator (ones-vector
    matmul over P^T = cross-partition sum on the PE) completes after 1/5 of
    the logits, so recip/broadcast/normalize (DVE+GPSIMD) hide under the
    next slice's logits.
  - P8 = e4m3(P^T * 128/rowsum) on DVE (the 128 keeps flat-attention rows
    out of fp8-subnormal flush); v^T evacuated as e4m3 hi + residual lo.
  - AV in fp8 DoubleRow pairs: out = sum_j vhi[j]^T P8[j] + vlo[j]^T P8[j]
    with (2j,2j+1) m-tile pairs per instruction - ~2x PE throughput vs fp16
    at fp16-grade v accuracy (end-to-end rel err ~1.4e-2 vs the 2e-2 gate).
  - The HW cadence of a matmul-chain -> PSUM -> evac -> consumer loop has
    ~200-700ns of dependency dead time per chain that neither deeper PSUM
    rotation nor wider evacs remove (measured), so the two samples' chains
    are INTERLEAVED: while sample 0's chain drains through exp/rowsum, the
    PE runs sample 1's independent chain, and vice versa.
"""
import numpy as np
from contextlib import ExitStack

import concourse.bass as bass
import concourse.mybir as mybir
import concourse.tile as tile
from concourse import bacc
from concourse.bass import ds, ts
from concourse.bass_utils import run_bass_kernel_spmd

f32 = mybir.dt.float32
fp16 = mybir.dt.float16
bf16 = mybir.dt.bfloat16
fp8 = mybir.dt.float8e4

B, C, H, W = 16, 256, 48, 48
N = H * W                      # 2304
NCORES = 8
SPC = B // NCORES              # samples per core
NT = N // 128                  # 18 m-tiles
M_SLICES = [(0, 512), (512, 512), (1024, 512), (1536, 512), (2048, 256)]
NSL = len(M_SLICES)
SHIFT = -120.0                 # softmax stabilizer: logits range [-193, 193]
PSCALE = 128.0                 # fp8 headroom scale for normalized P


def build(loop_n: int = 0, phases: str = "full", loop_xout: bool = False, lag: int = 1):
    nc = bacc.Bacc("TRN2", target_bir_lowering=False, debug=False)

    x_d = nc.dram_tensor("x", [SPC, C, N], fp16, kind="ExternalInput")
    wq_d = nc.dram_tensor("wqT", [C, C], fp16, kind="ExternalInput")
    wk_d = nc.dram_tensor("wkT", [C, C], fp16, kind="ExternalInput")
    wv_d = nc.dram_tensor("wvT", [C, C], fp16, kind="ExternalInput")
    pos_d = nc.dram_tensor("pos", [C, N], fp16, kind="ExternalInput")
    bq_d = nc.dram_tensor("bq", [2, 128, 1], f32, kind="ExternalInput")
    bk_d = nc.dram_tensor("bk", [2, 128, 1], f32, kind="ExternalInput")
    bv_d = nc.dram_tensor("bv", [2, 128, 1], f32, kind="ExternalInput")
    out_d = nc.dram_tensor("out", [SPC, C, N], f32, kind="ExternalOutput")

    with tile.TileContext(nc) as tc, ExitStack() as ctx:
        const = ctx.enter_context(tc.tile_pool(name="const", bufs=1))
        sb = ctx.enter_context(tc.tile_pool(name="sb", bufs=2))
        sb1 = ctx.enter_context(tc.tile_pool(name="sb1", bufs=1))
        ps = ctx.enter_context(tc.tile_pool(name="ps", bufs=1, space="PSUM"))

        wq = [const.tile([128, C], fp16, tag=f"wq{cc}", name=f"wq{cc}") for cc in range(2)]
        wk = [const.tile([128, C], fp16, tag=f"wk{cc}", name=f"wk{cc}") for cc in range(2)]
        wv = [const.tile([128, C], fp16, tag=f"wv{cc}", name=f"wv{cc}") for cc in range(2)]
        for cc in range(2):
            nc.gpsimd.dma_start(wq[cc][:], wq_d.ap()[ds(cc * 128, 128)])
            nc.gpsimd.dma_start(wk[cc][:], wk_d.ap()[ds(cc * 128, 128)])
            nc.gpsimd.dma_start(wv[cc][:], wv_d.ap()[ds(cc * 128, 128)])
        pos = [const.tile([128, N], fp16, tag=f"pos{cc}", name=f"pos{cc}") for cc in range(2)]
        for cc in range(2):
            nc.scalar.dma_start(pos[cc][:, 0:1152], pos_d.ap()[ds(cc * 128, 128), ds(0, 1152)])
            nc.scalar.dma_start(pos[cc][:, 1152:N], pos_d.ap()[ds(cc * 128, 128), ds(1152, N - 1152)])
        shift_sb = const.tile([128, 1], f32)
        nc.gpsimd.memset(shift_sb[:], SHIFT)
        ones_sb = const.tile([128, 1], bf16)
        nc.gpsimd.memset(ones_sb[:], 1.0)
        bq_sb = const.tile([128, 2], f32)
        bk_sb = const.tile([128, 2], f32)
        bv_sb = const.tile([128, 2], f32)
        for ot in range(2):
            nc.sync.dma_start(bq_sb[:, ds(ot, 1)], bq_d.ap()[ot])
            nc.sync.dma_start(bk_sb[:, ds(ot, 1)], bk_d.ap()[ot])
            nc.sync.dma_start(bv_sb[:, ds(ot, 1)], bv_d.ap()[ot])

        pre_x = None
        if loop_xout:
            pre_x = {}
            for s in range(SPC):
                for cc in range(2):
                    xt = const.tile([128, N], fp16, tag=f"px{s}{cc}", name=f"px{s}{cc}")
                    nc.sync.dma_start(xt[:], x_d.ap()[s, ds(cc * 128, 128)])
                    pre_x[(s, cc)] = xt

        def body(rep):
            # ---- load x for both samples ----
            xs = []
            for s in range(SPC):
                xc = []
                for cc in range(2):
                    if pre_x is not None:
                        xc.append(pre_x[(s, cc)])
                        continue
                    xt = sb.tile([128, N], fp16, tag=f"x{cc}", name=f"x{cc}_{rep}_{s}")
                    eng = nc.sync if cc == 0 else nc.gpsimd
                    eng.dma_start(xt[:, 0:1152], x_d.ap()[s, ds(cc * 128, 128), ds(0, 1152)])
                    eng.dma_start(xt[:, 1152:N], x_d.ap()[s, ds(cc * 128, 128), ds(1152, N - 1152)])
                    xc.append(xt)
                xs.append(xc)

            # ---- projections, s0/s1 chains interleaved ----
            q, k, vhi, vlo = {}, {}, {}, {}
            for s in range(SPC):
                for pname in ("q", "k"):
                    for ot in range(2):
                        (q if pname == "q" else k).setdefault(s, {})[ot] = sb.tile(
                            [128, N], fp16, tag=f"{pname}{ot}",
                            name=f"{pname}{ot}_{rep}_{s}")
                vhi[s] = sb.tile([128, NT, C], fp8, tag="vhi", name=f"vhi_{rep}_{s}")
                vlo[s] = sb.tile([128, NT, C], fp8, tag="vlo", name=f"vlo_{rep}_{s}")

            for pname, wt, bias in (("q", wq, bq_sb), ("k", wk, bk_sb)):
                dstd = q if pname == "q" else k
                for ot in range(2):
                    for mo, mw in M_SLICES:
                        for s in range(SPC):
                            pj = ps.tile([128, 512], f32, tag="L", bufs=4,
                                         name=f"pj_{rep}_{s}_{pname}{ot}_{mo}")
                            for cc in range(2):
                                nc.tensor.matmul(
                                    pj[:, :mw],
                                    wt[cc][:, ds(ot * 128, 128)],
                                    xs[s][cc][:, ds(mo, mw)],
                                    start=(cc == 0), stop=(cc == 1),
                                )
                            nc.scalar.activation(
                                dstd[s][ot][:, ds(mo, mw)], pj[:, :mw],
                                mybir.ActivationFunctionType.Identity,
                                bias=bias[:, ds(ot, 1)], scale=1.0,
                            )
            for nt in range(NT):
                for s in range(SPC):
                    pv = ps.tile([128, 512], f32, tag="L", bufs=4,
                                 name=f"pv_{rep}_{s}_{nt}")
                    for cc in range(2):
                        nc.tensor.matmul(
                            pv[:, :C],
                            xs[s][cc][:, ds(nt * 128, 128)],
                            wv[cc][:],
                            start=(cc == 0), stop=(cc == 1),
                        )
                    nc.scalar.copy(vhi[s][:, nt], pv[:, :C])
                    nc.vector.tensor_tensor(vlo[s][:, nt], pv[:, :C], vhi[s][:, nt],
                                            mybir.AluOpType.subtract)

            A_ch = {s: [q[s][0], q[s][1], pos[0], pos[1]] for s in range(SPC)}
            B_ch = {s: [k[s][0], k[s][1], q[s][0], q[s][1]] for s in range(SPC)}
            rs_all = [ps.tile([64, 512], f32, tag=f"rs{s}", bufs=1,
                              name=f"rs{s}_{rep}") for s in range(SPC)]

            def emit_slice(mi):
                """Logits+exp+rowsum for slice mi of BOTH samples, chains
                interleaved; then recip/broadcast/normalize per sample.
                Returns p8 per sample."""
                mo, mw = M_SLICES[mi]
                Pt = {s: sb1.tile([128, NT, 512], bf16, tag="P", bufs=3,
                                  name=f"P{mi}_{rep}_{s}") for s in range(SPC)}
                rs_ps = {s: rs_all[s][ds(32 * (mi % 2), 1), :mw] for s in range(SPC)}
                pend_rs = []
                for mt in range(NT):
                    for s in range(SPC):
                        lp = ps.tile([128, 512], f32, tag="L", bufs=4,
                                     name=f"lp_{rep}_{s}_{mt}_{mi}")
                        for ci in range(4):
                            nc.tensor.matmul(
                                lp[:, :mw],
                                B_ch[s][ci][:, ds(mt * 128, 128)],
                                A_ch[s][ci][:, ds(mo, mw)],
                                start=(ci == 0), stop=(ci == 3),
                            )
                        nc.scalar.activation(
                            Pt[s][:, mt, :mw], lp[:, :mw],
                            mybir.ActivationFunctionType.Exp,
                            bias=shift_sb[:], scale=1.0,
                        )
                        if phases != "logits":
                            pend_rs.append((s, mt))
                    # lagged rowsums: consume exps issued `lag` m-tiles ago so
                    # the PE never waits on the exp it just queued
                    while len(pend_rs) > SPC * lag:
                        ss, smt = pend_rs.pop(0)
                        nc.tensor.matmul(
                            rs_ps[ss], ones_sb[:], Pt[ss][:, smt, :mw],
                            start=(smt == 0), stop=(smt == NT - 1),
                        )
                for ss, smt in pend_rs:
                    nc.tensor.matmul(
                        rs_ps[ss], ones_sb[:], Pt[ss][:, smt, :mw],
                        start=(smt == 0), stop=(smt == NT - 1),
                    )
                if phases == "logits":
                    return None

                p8 = {}
                for s in range(SPC):
                    rr = sb1.tile([1, 512], f32, tag=f"rr{s}", bufs=2,
                                  name=f"rr{mi}_{rep}_{s}")
                    nc.vector.reciprocal(rr[:, :mw], rs_ps[s])
                    nc.vector.tensor_scalar_mul(rr[:, :mw], rr[:, :mw], PSCALE)
                    rb = sb1.tile([128, 512], f32, tag=f"rb{s}", bufs=2,
                                  name=f"rb{mi}_{rep}_{s}")
                    nc.gpsimd.partition_broadcast(rb[:, :mw], rr[:, :mw])
                    p8[s] = sb1.tile([128, NT, 512], fp8, tag="p8", bufs=3,
                                     name=f"p8{mi}_{rep}_{s}")
                    for mt in range(NT):
                        nc.vector.tensor_tensor(
                            p8[s][:, mt, :mw], Pt[s][:, mt, :mw], rb[:, :mw],
                            mybir.AluOpType.mult,
                        )
                return p8

            def emit_av(mi, p8):
                mo, mw = M_SLICES[mi]
                for ct in range(2):
                    for s in range(SPC):
                        po = ps.tile([128, 512], f32, tag="po", bufs=2,
                                     name=f"po_{rep}_{s}_{mi}_{ct}")
                        for half, vv in ((0, vhi[s]), (1, vlo[s])):
                            for j in range(NT // 2):
                                nc.tensor.matmul(
                                    po[:, :mw],
                                    vv[:, ds(2 * j, 2), ds(ct * 128, 128)],
                                    p8[s][:, ds(2 * j, 2), :mw],
                                    start=(half == 0 and j == 0),
                                    stop=(half == 1 and j == NT // 2 - 1),
                                    perf_mode=mybir.MatmulPerfMode.DoubleRow,
                                )
                        oe = sb.tile([128, 512], f32, tag="oe", bufs=2,
                                     name=f"oe_{rep}_{s}_{mi}_{ct}")
                        nc.scalar.activation(
                            oe[:, :mw], po[:, :mw],
                            mybir.ActivationFunctionType.Identity,
                            bias=bv_sb[:, ds(ct, 1)], scale=1.0 / PSCALE,
                        )
                        dma_eng = nc.sync if ct == 0 else nc.gpsimd
                        dma_eng.dma_start(
                            out_d.ap()[s, ds(ct * 128, 128), ds(mo, mw)],
                            oe[:, :mw],
                        )

            if phases in ("logits", "noav"):
                for mi in range(NSL):
                    emit_slice(mi)
                return
            # AV(mi) emitted after logits(mi+1): PE has slice mi+1's chains
            # while DVE normalizes slice mi.
            av_pend = None
            for mi in range(NSL):
                p8 = emit_slice(mi)
                if av_pend is not None:
                    av_pend()
                av_pend = (lambda mi=mi, p8=p8: emit_av(mi, p8))
            av_pend()

        if loop_n:
            with tc.For_i(0, loop_n, 1):
                body(0)
        else:
            body(0)
    nc.compile()
    return nc


_CACHE = {}


def _get_nc(loop_n: int = 0, phases: str = "full", loop_xout: bool = False, lag: int = 1):
    key = (loop_n, phases, loop_xout, lag)
    if key not in _CACHE:
        _CACHE[key] = build(loop_n, phases, loop_xout, lag)
    return _CACHE[key]


def _make_in_maps(x, Wq, bq, Wk, bk, Wv, bv, rel_h, rel_w):
    f = np.float32
    xr = np.asarray(x, dtype=f).reshape(B, C, N).astype(np.float16)
    pos = (np.asarray(rel_h, dtype=f) + np.asarray(rel_w, dtype=f)).reshape(C, N).astype(np.float16)
    wqT = np.ascontiguousarray(np.asarray(Wq, dtype=f).T).astype(np.float16)
    wkT = np.ascontiguousarray(np.asarray(Wk, dtype=f).T).astype(np.float16)
    wvT = np.ascontiguousarray(np.asarray(Wv, dtype=f).T).astype(np.float16)
    bqr = np.ascontiguousarray(np.asarray(bq, dtype=f).reshape(2, 128, 1))
    bkr = np.ascontiguousarray(np.asarray(bk, dtype=f).reshape(2, 128, 1))
    bvr = np.ascontiguousarray(np.asarray(bv, dtype=f).reshape(2, 128, 1))
    maps = []
    for i in range(NCORES):
        maps.append({
            "x": np.ascontiguousarray(xr[i * SPC:(i + 1) * SPC]),
            "wqT": wqT, "wkT": wkT, "wvT": wvT, "pos": pos,
            "bq": bqr, "bk": bkr, "bv": bvr,
        })
    return maps


def kernel(x, Wq, bq, Wk, bk, Wv, bv, rel_h, rel_w):
    nc = _get_nc()
    in_maps = _make_in_maps(x, Wq, bq, Wk, bk, Wv, bv, rel_h, rel_w)
    res = run_bass_kernel_spmd(nc, in_maps, core_ids=list(range(NCORES)))
    out = np.concatenate([r["out"] for r in res.results], axis=0)
    return np.ascontiguousarray(out.reshape(B, C, H, W).astype(np.float32))
